# revision 44
# baseline (speedup 1.0000x reference)
"""Longformer sliding-window attention (W=128) on 8 Trainium2 NeuronCores.

Strategy (fp16 compute, f32 scores, batch*head sharding):
  - 24 (b,h) slices across 8 cores, 3 per core; window attention is local
    per slice, so no collectives.
  - Host pre-casts q/k/v to fp16, pre-transposes q/k to [d, seq] with two
    bh slices packed along d (fills the 128 SBUF partitions), and lays v
    out in the device's [x, block, d] tiling, so every DMA is a plain
    contiguous copy with multi-KB descriptors.
  - Per query block n (128 queries), both packed bh slices interleaved:
      PE : QK matmuls for the two bh emitted back-to-back - they occupy
           disjoint row groups (rows 0-63 / 64-127) and run concurrently;
           scores land in one paired PSUM tile [128, 2, 512] f32.
           Edge blocks add a -30720 additive mask matmul (identity @ mask)
           so out-of-range slots become exp(..) = 0.
      ACT: one exp over both slices: em = exp(s/8 - 6) -> fp16 rows,
           written straight into the DRAM-bound staging tile.
      PE : transpose the 3 valid 128-chunks of each em row block -> PSUM
           (fp16), then one band-mask-predicated DVE copy into pre-zeroed
           SBUF buffers (kills the out-of-window triangles for free).
      PE : ctx[128, 2, 64] += emT_c.T @ v_c (3 accumulating matmuls per
           slice); one DVE cast copies both ctx slices out.
  - Outputs are unnormalized (em rows + ctx); the host extracts the 257
    diagonal band (probs[x,t] = em[n, x, x+t]), computes row sums over the
    band, and normalizes probs and ctx. End-to-end abs-rel error ~1e-3.
"""

import numpy as np

import concourse.bacc as bacc
import concourse.mybir as mybir
import concourse.tile as tile
from concourse.bass_utils import run_bass_kernel_spmd

F16 = mybir.dt.float16
F32 = mybir.dt.float32
U16 = mybir.dt.uint16

NCORES = 8
B, H, S, D = 2, 12, 4096, 64
W = 128
NB = S // W          # 32 query blocks per bh
T = 2 * W + 1        # 257 band width
BHPC = (B * H) // NCORES  # 3 bh slices per core
NPAIR = (BHPC + 1) // 2   # bh pairs packed along d into 128 partitions
NEG = -30720.0
SCALE = 0.125        # 1/sqrt(D)
EBIAS = -6.0         # uniform exp shift, cancels in softmax
PS = 512             # paired-PSUM per-slice pitch (bank aligned)


def _np_constants():
    x = np.arange(W)[:, None]
    y = np.arange(3 * W)[None, :]
    band = (y - x >= 0) & (y - x <= 2 * W)
    first = band & (y >= W)
    last = band & (y < 2 * W)
    # additive -30720 masks for the two edge blocks (define full PSUM rows)
    masks = np.stack(
        [np.where(m, 0.0, NEG) for m in (first, last)]
    ).astype(np.float16)  # [2, 128, 384]
    # transposed band-validity mask for the predicated emT copies:
    # emT layout is [y_local, c*W + x] = em[x, c*W + y_local]
    yl = np.arange(W)[:, None]
    xx = np.arange(W)[None, :]
    mt = np.zeros((W, 3 * W), np.float16)
    for c in range(3):
        yg = c * W + yl
        mt[:, c * W : (c + 1) * W] = (yg - xx >= 0) & (yg - xx <= 2 * W)
    ident = np.eye(W, dtype=np.float16)
    ebias = np.full((W, 1), EBIAS, dtype=np.float32)
    return masks, mt, ident, ebias


def _build_program(BHPC=BHPC, NB=NB):
    S = NB * W
    NPAIR = (BHPC + 1) // 2
    nc = bacc.Bacc("TRN2", target_bir_lowering=False, debug=False)

    # host-prepared fp16 inputs (see _prep_core_inputs for layouts)
    qt_i = nc.dram_tensor("qt", [NPAIR, 2 * D, S], F16, kind="ExternalInput").ap()
    kt_i = nc.dram_tensor("kt", [NPAIR, 2 * D, S], F16, kind="ExternalInput").ap()
    v_i = nc.dram_tensor("v", [BHPC, W, NB, D], F16, kind="ExternalInput").ap()
    # unnormalized outputs in device tiling; host reorders + normalizes
    ctx_o = nc.dram_tensor("ctx", [BHPC, W, NB, D], F16, kind="ExternalOutput").ap()
    em_o = nc.dram_tensor("emn", [BHPC, W, NB, 3 * W], F16, kind="ExternalOutput").ap()

    masks_np, mt_np, ident_np, ebias_np = _np_constants()
    masks_d = nc.inline_tensor(masks_np, "masks_c").ap()
    mt_d = nc.inline_tensor(mt_np, "mt_c").ap()
    ident_d = nc.inline_tensor(ident_np, "ident_c").ap()
    ebias_d = nc.inline_tensor(ebias_np, "ebias_c").ap()

    with tile.TileContext(nc) as tc:
        with (
            tc.tile_pool(name="const", bufs=1) as constp,
            tc.tile_pool(name="qt", bufs=4) as qtp,
            tc.tile_pool(name="kt", bufs=4) as ktp,
            tc.tile_pool(name="vp", bufs=3) as vp,
            tc.tile_pool(name="emn", bufs=1) as emnp,
            tc.tile_pool(name="emt", bufs=4) as emtp,
            tc.tile_pool(name="ctxs", bufs=1) as ctxsp,
            tc.tile_pool(name="ps_s", bufs=2, space="PSUM") as ps_s,
            tc.tile_pool(name="ps_t", bufs=2, space="PSUM") as ps_t,
            tc.tile_pool(name="ps_c", bufs=2, space="PSUM") as ps_c,
        ):
            masks = constp.tile([W, 2, 3 * W], F16, tag="masks")
            nc.sync.dma_start(masks[:], masks_d.rearrange("m x y -> x m y"))
            maskT = constp.tile([W, 3 * W], F16, tag="maskT")
            nc.sync.dma_start(maskT[:], mt_d)
            ident = constp.tile([W, W], F16, tag="ident")
            nc.sync.dma_start(ident[:], ident_d)
            ebias = constp.tile([W, 1], F32, tag="ebias")
            nc.sync.dma_start(ebias[:], ebias_d)

            emn_all = emnp.tile([W, BHPC, NB, 3 * W], F16, tag="emn")
            ctx_all = ctxsp.tile([W, BHPC, NB, D], F16, tag="ctxs")

            # paired emT buffers (alternated by block parity) whose
            # invalid-triangle regions are zeroed once and never rewritten
            # (the in-loop copies are predicated on the band mask)
            emT_bufs = []
            for i in range(4):
                t = emtp.tile([W, 2, PS], F16, tag="emt", name=f"emT{i}")
                nc.vector.memset(t[:], 0.0)
                emT_bufs.append(t)

            # flat [W, BHPC*NB, F] views for strided pair APs
            emn_flat = emn_all[:].rearrange("x b n y -> x (b n) y")
            ctx_flat = ctx_all[:].rearrange("x b n d -> x (b n) d")

            qT = {}
            kT = {}
            HB = NB // 2          # blocks per half
            KA = (HB + 1) * W     # kT first-half width (one block overlap)
            KBO = (HB - 1) * W    # kT second-half global col offset

            def load_pair(p, eng):
                # k/q split into half-range tiles so the first half's bytes
                # drain first (FIFO ring) and compute starts early
                qA = qtp.tile([2 * D, HB * W], F16, tag="qt", name=f"qA{p}")
                qB = qtp.tile([2 * D, HB * W], F16, tag="qt", name=f"qB{p}")
                kA = ktp.tile([2 * D, KA], F16, tag="kt", name=f"kA{p}")
                kB = ktp.tile([2 * D, KA], F16, tag="kt", name=f"kB{p}")
                eng.dma_start(kA[:], kt_i[p][:, :KA])
                eng.dma_start(qA[:], qt_i[p][:, : HB * W])
                eng.dma_start(kB[:], kt_i[p][:, KBO : KBO + KA])
                eng.dma_start(qB[:], qt_i[p][:, HB * W :])
                qT[p] = (qA, qB)
                kT[p] = (kA, kB)

            def qk_slices(p, nn, c_lo, c_hi, dlo):
                """(lhsT, rhs) for block nn from the half-range tiles."""
                qA, qB = qT[p]
                kA, kB = kT[p]
                if nn < HB:
                    q_ap = qA[dlo : dlo + D, nn * W : (nn + 1) * W]
                    k_ap = kA[dlo : dlo + D,
                              (nn - 1 + c_lo) * W : (nn - 1 + c_hi) * W]
                else:
                    q_ap = qB[dlo : dlo + D,
                              (nn - HB) * W : (nn - HB + 1) * W]
                    lo = (nn - 1 + c_lo) * W - KBO
                    k_ap = kB[dlo : dlo + D, lo : lo + (c_hi - c_lo) * W]
                return q_ap, k_ap

            def pair_ap(flat, bh_nns):
                idxs = [bh * NB + nn for bh, nn in bh_nns]
                if len(idxs) == 1:
                    return flat[:, idxs[0] : idxs[0] + 1]
                step = idxs[1] - idxs[0]
                return flat[:, idxs[0] : idxs[1] + 1 : step]

            # each group runs `niter` iterations; slice i of the d-packed
            # qt/kt handles block j + off at PE rows [64i, 64i+64) -- the
            # odd bh self-pairs its first and second half of blocks
            if BHPC == 3 and NB % 2 == 0:
                groups = [
                    (NB, [(0, 0), (1, 0)]),
                    (NB // 2, [(2, 0), (2, NB // 2)]),
                ]
            else:
                groups = [
                    (NB, [(bh, 0) for bh in range(2 * p, min(2 * p + 2, BHPC))])
                    for p in range(NPAIR)
                ]

            for p, (niter, slices) in enumerate(groups):
                g = len(slices)
                # startup loads go out on the ACT HWDGE queue so they don't
                # serialize behind the const/v loads on SP
                load_pair(p, nc.scalar if p == 0 else nc.sync)
                # v loads: head blocks first so early PVs aren't starved
                v_sb = {}
                vbhs = sorted({bh for bh, _ in slices})
                for bh in vbhs:
                    v_sb[bh] = vp.tile([W, NB, D], F16, tag="v", name=f"v{bh}")
                    nc.sync.dma_start(v_sb[bh][:, :8], v_i[bh][:, :8])
                for bh in vbhs:
                    nc.sync.dma_start(v_sb[bh][:, 8:], v_i[bh][:, 8:])

                for j in range(niter):
                    nns = [j + off for _, off in slices]
                    cl = [1 if nn == 0 else 0 for nn in nns]
                    ch_ = [2 if nn == NB - 1 else 3 for nn in nns]

                    # QK matmuls for the two slices are emitted back-to-back:
                    # disjoint PE row groups run concurrently on the array
                    psum_s = ps_s.tile([W, 2, PS], F32, tag="ps_s", name="pss")
                    for i, (bh, _) in enumerate(slices):
                        nn, c_lo, c_hi = nns[i], cl[i], ch_[i]
                        dlo = i * D
                        q_ap, k_ap = qk_slices(p, nn, c_lo, c_hi, dlo)
                        qk_args = (psum_s[:, i, c_lo * W : c_hi * W], q_ap, k_ap)
                        if c_hi - c_lo < 3:
                            # edge block: additive mask defines the columns
                            # the QK matmul does not cover
                            nc.tensor.matmul(
                                psum_s[:, i, : 3 * W], ident[:],
                                masks[:, 0 if nn == 0 else 1, :],
                                start=True, stop=False,
                            )
                            nc.tensor.matmul(*qk_args, start=False, stop=True)
                        else:
                            nc.tensor.matmul(*qk_args, start=True, stop=True)

                    # one exp over both slices, written straight into the
                    # DRAM-bound staging rows (triangles unmasked; the host
                    # band extraction skips them)
                    bh_nns = [(bh, nns[i]) for i, (bh, _) in enumerate(slices)]
                    nc.scalar.activation(
                        pair_ap(emn_flat, bh_nns), psum_s[:, :g, : 3 * W],
                        mybir.ActivationFunctionType.Exp,
                        bias=ebias[:], scale=SCALE,
                    )

                    # transpose all chunks of both slices into one PSUM
                    # bank (edge slices transpose their zeroed chunk too so
                    # the mask multiply below reads fully-written PSUM)
                    pt = ps_t.tile([W, 2, PS], F16, tag="ps_t", name="pt")
                    for i, (bh, _) in enumerate(slices):
                        nn = nns[i]
                        for c in range(3):
                            nc.tensor.transpose(
                                pt[:, i, c * W : (c + 1) * W],
                                emn_all[:, bh, nn, c * W : (c + 1) * W],
                                ident[:],
                            )
                    # one band-mask multiply zeroes the out-of-window
                    # triangles while moving emT to SBUF
                    emT = emT_bufs[j % 4]
                    nc.vector.tensor_mul(
                        emT[:, :g, : 3 * W],
                        pt[:, :g, : 3 * W],
                        maskT[:].unsqueeze(1).broadcast_to([W, g, 3 * W]),
                    )

                    psum_ctx = ps_c.tile([W, 2, D], F32, tag="ps_c", name="pc")
                    for i, (bh, _) in enumerate(slices):
                        nn, c_lo, c_hi = nns[i], cl[i], ch_[i]
                        for c in range(c_lo, c_hi):
                            nc.tensor.matmul(
                                psum_ctx[:, i, :],
                                emT[:, i, c * W : (c + 1) * W],
                                v_sb[bh][:, nn - 1 + c, :],
                                start=(c == c_lo), stop=(c == c_hi - 1),
                            )
                    # unnormalized ctx for both slices in one cast copy;
                    # host divides by the band row sums
                    nc.vector.tensor_copy(
                        pair_ap(ctx_flat, bh_nns), psum_ctx[:, :g, :]
                    )

                    # drain finished 4-block output ranges while computing
                    if (j + 1) % 4 == 0:
                        for bh, off in slices:
                            hs = slice(off + j - 3, off + j + 1)
                            nc.sync.dma_start(
                                ctx_o[bh][:, hs], ctx_all[:, bh, hs]
                            )
                            nc.sync.dma_start(
                                em_o[bh][:, hs], emn_all[:, bh, hs]
                            )

    nc.compile()
    return nc


_PROGRAM = None


def _get_program():
    global _PROGRAM
    if _PROGRAM is None:
        _PROGRAM = _build_program()
    return _PROGRAM


# host-side diagonal band extraction index: band[x, t] = em_row[x, x + t]
_BAND_IDX = (np.arange(W)[:, None] + np.arange(T)[None, :])  # [128, 257]


def _extract_band(em_raw):
    """[nbh, W, NB, 384] fp16 device tiling -> [nbh, S, 257] band."""
    nbh = em_raw.shape[0]
    nb = em_raw.shape[2]
    idx = _BAND_IDX[:, None, :].astype(np.intp)  # [W, 1, T]
    band = np.take_along_axis(
        em_raw, np.broadcast_to(idx, (nbh, W, nb, T)), axis=3
    )  # [nbh, W, nb, T]
    return band.transpose(0, 2, 1, 3).reshape(nbh, nb * W, T)


def _prep_core_inputs(qf, kf, vf, lo):
    """Host-side fp16 cast + [d, seq] transpose + bh-pair packing."""
    q16 = qf[lo : lo + BHPC].astype(np.float16)
    k16 = kf[lo : lo + BHPC].astype(np.float16)
    v16 = vf[lo : lo + BHPC].astype(np.float16)
    qt = np.zeros((NPAIR, 2 * D, S), np.float16)
    kt = np.zeros((NPAIR, 2 * D, S), np.float16)
    for bh in range(BHPC):
        p, half = bh // 2, bh % 2
        qt[p, half * D : (half + 1) * D] = q16[bh].T
        kt[p, half * D : (half + 1) * D] = k16[bh].T
    if BHPC % 2 == 1:
        # the odd bh self-pairs (slice 1 runs its second half of blocks)
        qt[NPAIR - 1, D : 2 * D] = q16[BHPC - 1].T
        kt[NPAIR - 1, D : 2 * D] = k16[BHPC - 1].T
    vdev = np.ascontiguousarray(
        v16.reshape(BHPC, NB, W, D).transpose(0, 2, 1, 3)
    )
    return {
        "qt": np.ascontiguousarray(qt),
        "kt": np.ascontiguousarray(kt),
        "v": vdev,
    }


def kernel(q, k, v, numeric_embedding_manager=None, **_unused):
    nc = _get_program()
    qf = np.asarray(q, dtype=np.float32).reshape(B * H, S, D)
    kf = np.asarray(k, dtype=np.float32).reshape(B * H, S, D)
    vf = np.asarray(v, dtype=np.float32).reshape(B * H, S, D)

    in_maps = [
        _prep_core_inputs(qf, kf, vf, i * BHPC) for i in range(NCORES)
    ]
    res = run_bass_kernel_spmd(nc, in_maps, core_ids=list(range(NCORES)))

    ctx_raw = np.concatenate(
        [res.results[i]["ctx"] for i in range(NCORES)], axis=0
    ).astype(np.float32)  # [24, W, NB, D]
    ctx_raw = ctx_raw.transpose(0, 2, 1, 3).reshape(B * H, S, D)
    em_raw = np.concatenate(
        [res.results[i]["emn"] for i in range(NCORES)], axis=0
    )
    band = _extract_band(em_raw).astype(np.float32)
    rn = 1.0 / band.sum(axis=2, keepdims=True)
    probs = band * rn
    ctx = (ctx_raw * rn).reshape(B, H, S, D)
    return ctx, probs


# revision 45
# speedup vs baseline: 1.0162x; 1.0162x over previous
"""Longformer sliding-window attention (W=128) on 8 Trainium2 NeuronCores.

Strategy (fp16 compute, f32 scores, batch*head sharding):
  - 24 (b,h) slices across 8 cores, 3 per core; window attention is local
    per slice, so no collectives.
  - Host pre-casts q/k/v to fp16, pre-transposes q/k to [d, seq] with two
    bh slices packed along d (fills the 128 SBUF partitions), and lays v
    out in the device's [x, block, d] tiling, so every DMA is a plain
    contiguous copy with multi-KB descriptors.
  - Per query block n (128 queries), both packed bh slices interleaved:
      PE : QK matmuls for the two bh emitted back-to-back - they occupy
           disjoint row groups (rows 0-63 / 64-127) and run concurrently;
           scores land in one paired PSUM tile [128, 2, 512] f32.
           Edge blocks add a -30720 additive mask matmul (identity @ mask)
           so out-of-range slots become exp(..) = 0.
      ACT: one exp over both slices: em = exp(s/8 - 6) -> fp16 rows,
           written straight into the DRAM-bound staging tile.
      PE : transpose the 128-chunks of each em row block -> PSUM (fp16),
           then one DVE multiply by the 0/1 transposed band mask moves
           them to SBUF and zeroes the out-of-window triangles.
      PE : ctx[128, 2, 64] += emT_c.T @ v_c (3 accumulating matmuls per
           slice); one DVE cast copies both ctx slices out.
  - Outputs are unnormalized (em rows + ctx); the host extracts the 257
    diagonal band (probs[x,t] = em[n, x, x+t]), computes row sums over the
    band, and normalizes probs and ctx. End-to-end abs-rel error ~1e-3.
"""

import numpy as np

import concourse.bacc as bacc
import concourse.mybir as mybir
import concourse.tile as tile
from concourse.bass_utils import run_bass_kernel_spmd

F16 = mybir.dt.float16
F32 = mybir.dt.float32

NCORES = 8
B, H, S, D = 2, 12, 4096, 64
W = 128
NB = S // W          # 32 query blocks per bh
T = 2 * W + 1        # 257 band width
BHPC = (B * H) // NCORES  # 3 bh slices per core
NPAIR = (BHPC + 1) // 2   # bh pairs packed along d into 128 partitions
NEG = -30720.0
SCALE = 0.125        # 1/sqrt(D)
EBIAS = -6.0         # uniform exp shift, cancels in softmax
PS = 512             # paired-PSUM per-slice pitch (bank aligned)


def _np_constants():
    x = np.arange(W)[:, None]
    y = np.arange(3 * W)[None, :]
    band = (y - x >= 0) & (y - x <= 2 * W)
    first = band & (y >= W)
    last = band & (y < 2 * W)
    # additive -30720 masks for the two edge blocks (define full PSUM rows)
    masks = np.stack(
        [np.where(m, 0.0, NEG) for m in (first, last)]
    ).astype(np.float16)  # [2, 128, 384]
    # transposed 0/1 band-validity mask applied to the emT chunks:
    # emT layout is [y_local, c*W + x] = em[x, c*W + y_local]
    yl = np.arange(W)[:, None]
    xx = np.arange(W)[None, :]
    mt = np.zeros((W, 3 * W), np.float16)
    for c in range(3):
        yg = c * W + yl
        mt[:, c * W : (c + 1) * W] = (yg - xx >= 0) & (yg - xx <= 2 * W)
    ident = np.eye(W, dtype=np.float16)
    ebias = np.full((W, 1), EBIAS, dtype=np.float32)
    return masks, mt, ident, ebias


def _build_program(BHPC=BHPC, NB=NB):
    S = NB * W
    NPAIR = (BHPC + 1) // 2
    nc = bacc.Bacc("TRN2", target_bir_lowering=False, debug=False)

    # host-prepared fp16 inputs (see _prep_core_inputs for layouts)
    qt_i = nc.dram_tensor("qt", [NPAIR, 2 * D, S], F16, kind="ExternalInput").ap()
    kt_i = nc.dram_tensor("kt", [NPAIR, 2 * D, S], F16, kind="ExternalInput").ap()
    v_i = nc.dram_tensor("v", [BHPC, W, NB, D], F16, kind="ExternalInput").ap()
    # unnormalized outputs in device tiling; host reorders + normalizes
    ctx_o = nc.dram_tensor("ctx", [BHPC, W, NB, D], F16, kind="ExternalOutput").ap()
    em_o = nc.dram_tensor("emn", [BHPC, W, NB, 3 * W], F16, kind="ExternalOutput").ap()

    masks_np, mt_np, ident_np, ebias_np = _np_constants()
    masks_d = nc.inline_tensor(masks_np, "masks_c").ap()
    mt_d = nc.inline_tensor(mt_np, "mt_c").ap()
    ident_d = nc.inline_tensor(ident_np, "ident_c").ap()
    ebias_d = nc.inline_tensor(ebias_np, "ebias_c").ap()

    with tile.TileContext(nc) as tc:
        with (
            tc.tile_pool(name="const", bufs=1) as constp,
            tc.tile_pool(name="qt", bufs=4) as qtp,
            tc.tile_pool(name="kt", bufs=4) as ktp,
            tc.tile_pool(name="vp", bufs=3) as vp,
            tc.tile_pool(name="emn", bufs=1) as emnp,
            tc.tile_pool(name="emt", bufs=4) as emtp,
            tc.tile_pool(name="ctxs", bufs=1) as ctxsp,
            tc.tile_pool(name="ps_s", bufs=2, space="PSUM") as ps_s,
            tc.tile_pool(name="ps_t", bufs=2, space="PSUM") as ps_t,
            tc.tile_pool(name="ps_c", bufs=2, space="PSUM") as ps_c,
        ):
            masks = constp.tile([W, 2, 3 * W], F16, tag="masks")
            nc.sync.dma_start(masks[:], masks_d.rearrange("m x y -> x m y"))
            maskT = constp.tile([W, 3 * W], F16, tag="maskT")
            nc.sync.dma_start(maskT[:], mt_d)
            ident = constp.tile([W, W], F16, tag="ident")
            nc.sync.dma_start(ident[:], ident_d)
            ebias = constp.tile([W, 1], F32, tag="ebias")
            nc.sync.dma_start(ebias[:], ebias_d)

            emn_all = emnp.tile([W, BHPC, NB, 3 * W], F16, tag="emn")
            ctx_all = ctxsp.tile([W, BHPC, NB, D], F16, tag="ctxs")

            # paired emT buffers, rotated across iterations
            emT_bufs = []
            for i in range(4):
                t = emtp.tile([W, 2, PS], F16, tag="emt", name=f"emT{i}")
                nc.vector.memset(t[:], 0.0)
                emT_bufs.append(t)

            # flat [W, BHPC*NB, F] views for strided pair APs
            emn_flat = emn_all[:].rearrange("x b n y -> x (b n) y")
            ctx_flat = ctx_all[:].rearrange("x b n d -> x (b n) d")

            qT = {}
            kT = {}
            HB = NB // 2          # blocks per half
            KA = (HB + 1) * W     # kT first-half width (one block overlap)
            KBO = (HB - 1) * W    # kT second-half global col offset

            def load_pair(p, eng):
                # k/q split into half-range tiles so the first half's bytes
                # drain first (FIFO ring) and compute starts early
                qA = qtp.tile([2 * D, HB * W], F16, tag="qt", name=f"qA{p}")
                qB = qtp.tile([2 * D, HB * W], F16, tag="qt", name=f"qB{p}")
                kA = ktp.tile([2 * D, KA], F16, tag="kt", name=f"kA{p}")
                kB = ktp.tile([2 * D, KA], F16, tag="kt", name=f"kB{p}")
                eng.dma_start(kA[:], kt_i[p][:, :KA])
                eng.dma_start(qA[:], qt_i[p][:, : HB * W])
                eng.dma_start(kB[:], kt_i[p][:, KBO : KBO + KA])
                eng.dma_start(qB[:], qt_i[p][:, HB * W :])
                qT[p] = (qA, qB)
                kT[p] = (kA, kB)

            def qk_slices(p, nn, c_lo, c_hi, dlo):
                """(lhsT, rhs) for block nn from the half-range tiles."""
                qA, qB = qT[p]
                kA, kB = kT[p]
                if nn < HB:
                    q_ap = qA[dlo : dlo + D, nn * W : (nn + 1) * W]
                    k_ap = kA[dlo : dlo + D,
                              (nn - 1 + c_lo) * W : (nn - 1 + c_hi) * W]
                else:
                    q_ap = qB[dlo : dlo + D,
                              (nn - HB) * W : (nn - HB + 1) * W]
                    lo = (nn - 1 + c_lo) * W - KBO
                    k_ap = kB[dlo : dlo + D, lo : lo + (c_hi - c_lo) * W]
                return q_ap, k_ap

            def pair_ap(flat, bh_nns):
                idxs = [bh * NB + nn for bh, nn in bh_nns]
                if len(idxs) == 1:
                    return flat[:, idxs[0] : idxs[0] + 1]
                step = idxs[1] - idxs[0]
                return flat[:, idxs[0] : idxs[1] + 1 : step]

            # each group runs `niter` iterations; slice i of the d-packed
            # qt/kt handles block j + off at PE rows [64i, 64i+64) -- the
            # odd bh self-pairs its first and second half of blocks
            if BHPC == 3 and NB % 2 == 0:
                groups = [
                    (NB, [(0, 0), (1, 0)]),
                    (NB // 2, [(2, 0), (2, NB // 2)]),
                ]
            else:
                groups = [
                    (NB, [(bh, 0) for bh in range(2 * p, min(2 * p + 2, BHPC))])
                    for p in range(NPAIR)
                ]

            for p, (niter, slices) in enumerate(groups):
                g = len(slices)
                # startup loads go out on the ACT HWDGE queue so they don't
                # serialize behind the const/v loads on SP
                load_pair(p, nc.scalar if p == 0 else nc.sync)
                # v loads: head blocks first so early PVs aren't starved
                v_sb = {}
                vbhs = sorted({bh for bh, _ in slices})
                for bh in vbhs:
                    v_sb[bh] = vp.tile([W, NB, D], F16, tag="v", name=f"v{bh}")
                    nc.sync.dma_start(v_sb[bh][:, :8], v_i[bh][:, :8])
                for bh in vbhs:
                    nc.sync.dma_start(v_sb[bh][:, 8:], v_i[bh][:, 8:])

                for j in range(niter):
                    nns = [j + off for _, off in slices]
                    cl = [1 if nn == 0 else 0 for nn in nns]
                    ch_ = [2 if nn == NB - 1 else 3 for nn in nns]

                    # QK matmuls for the two slices are emitted back-to-back:
                    # disjoint PE row groups run concurrently on the array
                    psum_s = ps_s.tile([W, 2, PS], F32, tag="ps_s", name="pss")
                    for i, (bh, _) in enumerate(slices):
                        nn, c_lo, c_hi = nns[i], cl[i], ch_[i]
                        dlo = i * D
                        q_ap, k_ap = qk_slices(p, nn, c_lo, c_hi, dlo)
                        qk_args = (psum_s[:, i, c_lo * W : c_hi * W], q_ap, k_ap)
                        if c_hi - c_lo < 3:
                            # edge block: additive mask defines the columns
                            # the QK matmul does not cover
                            nc.tensor.matmul(
                                psum_s[:, i, : 3 * W], ident[:],
                                masks[:, 0 if nn == 0 else 1, :],
                                start=True, stop=False,
                            )
                            nc.tensor.matmul(*qk_args, start=False, stop=True)
                        else:
                            nc.tensor.matmul(*qk_args, start=True, stop=True)

                    # one exp over both slices, written straight into the
                    # DRAM-bound staging rows (triangles unmasked; the host
                    # band extraction skips them)
                    bh_nns = [(bh, nns[i]) for i, (bh, _) in enumerate(slices)]
                    nc.scalar.activation(
                        pair_ap(emn_flat, bh_nns), psum_s[:, :g, : 3 * W],
                        mybir.ActivationFunctionType.Exp,
                        bias=ebias[:], scale=SCALE,
                    )

                    # transpose all chunks of both slices into one PSUM
                    # bank (edge slices transpose their zeroed chunk too so
                    # the mask multiply below reads fully-written PSUM)
                    pt = ps_t.tile([W, 2, PS], F16, tag="ps_t", name="pt")
                    for i, (bh, _) in enumerate(slices):
                        nn = nns[i]
                        for c in range(3):
                            nc.tensor.transpose(
                                pt[:, i, c * W : (c + 1) * W],
                                emn_all[:, bh, nn, c * W : (c + 1) * W],
                                ident[:],
                            )
                    # one band-mask multiply zeroes the out-of-window
                    # triangles while moving emT to SBUF
                    emT = emT_bufs[j % 4]
                    nc.vector.tensor_mul(
                        emT[:, :g, : 3 * W],
                        pt[:, :g, : 3 * W],
                        maskT[:].unsqueeze(1).broadcast_to([W, g, 3 * W]),
                    )

                    psum_ctx = ps_c.tile([W, 2, D], F32, tag="ps_c", name="pc")
                    for i, (bh, _) in enumerate(slices):
                        nn, c_lo, c_hi = nns[i], cl[i], ch_[i]
                        for c in range(c_lo, c_hi):
                            nc.tensor.matmul(
                                psum_ctx[:, i, :],
                                emT[:, i, c * W : (c + 1) * W],
                                v_sb[bh][:, nn - 1 + c, :],
                                start=(c == c_lo), stop=(c == c_hi - 1),
                            )
                    # unnormalized ctx for both slices in one cast copy;
                    # host divides by the band row sums
                    nc.vector.tensor_copy(
                        pair_ap(ctx_flat, bh_nns), psum_ctx[:, :g, :]
                    )

                    # drain finished 4-block output ranges while computing
                    if (j + 1) % 4 == 0:
                        for bh, off in slices:
                            hs = slice(off + j - 3, off + j + 1)
                            nc.sync.dma_start(
                                ctx_o[bh][:, hs], ctx_all[:, bh, hs]
                            )
                            nc.sync.dma_start(
                                em_o[bh][:, hs], emn_all[:, bh, hs]
                            )

    nc.compile()
    return nc


_PROGRAM = None


def _get_program():
    global _PROGRAM
    if _PROGRAM is None:
        _PROGRAM = _build_program()
    return _PROGRAM


# host-side diagonal band extraction index: band[x, t] = em_row[x, x + t]
_BAND_IDX = (np.arange(W)[:, None] + np.arange(T)[None, :])  # [128, 257]


def _extract_band(em_raw):
    """[nbh, W, NB, 384] fp16 device tiling -> [nbh, S, 257] band."""
    nbh = em_raw.shape[0]
    nb = em_raw.shape[2]
    idx = _BAND_IDX[:, None, :].astype(np.intp)  # [W, 1, T]
    band = np.take_along_axis(
        em_raw, np.broadcast_to(idx, (nbh, W, nb, T)), axis=3
    )  # [nbh, W, nb, T]
    return band.transpose(0, 2, 1, 3).reshape(nbh, nb * W, T)


def _prep_core_inputs(qf, kf, vf, lo):
    """Host-side fp16 cast + [d, seq] transpose + bh-pair packing."""
    q16 = qf[lo : lo + BHPC].astype(np.float16)
    k16 = kf[lo : lo + BHPC].astype(np.float16)
    v16 = vf[lo : lo + BHPC].astype(np.float16)
    qt = np.zeros((NPAIR, 2 * D, S), np.float16)
    kt = np.zeros((NPAIR, 2 * D, S), np.float16)
    for bh in range(BHPC):
        p, half = bh // 2, bh % 2
        qt[p, half * D : (half + 1) * D] = q16[bh].T
        kt[p, half * D : (half + 1) * D] = k16[bh].T
    if BHPC % 2 == 1:
        # the odd bh self-pairs (slice 1 runs its second half of blocks)
        qt[NPAIR - 1, D : 2 * D] = q16[BHPC - 1].T
        kt[NPAIR - 1, D : 2 * D] = k16[BHPC - 1].T
    vdev = np.ascontiguousarray(
        v16.reshape(BHPC, NB, W, D).transpose(0, 2, 1, 3)
    )
    return {
        "qt": np.ascontiguousarray(qt),
        "kt": np.ascontiguousarray(kt),
        "v": vdev,
    }


def kernel(q, k, v, numeric_embedding_manager=None, **_unused):
    nc = _get_program()
    qf = np.asarray(q, dtype=np.float32).reshape(B * H, S, D)
    kf = np.asarray(k, dtype=np.float32).reshape(B * H, S, D)
    vf = np.asarray(v, dtype=np.float32).reshape(B * H, S, D)

    in_maps = [
        _prep_core_inputs(qf, kf, vf, i * BHPC) for i in range(NCORES)
    ]
    res = run_bass_kernel_spmd(nc, in_maps, core_ids=list(range(NCORES)))

    ctx_raw = np.concatenate(
        [res.results[i]["ctx"] for i in range(NCORES)], axis=0
    ).astype(np.float32)  # [24, W, NB, D]
    ctx_raw = ctx_raw.transpose(0, 2, 1, 3).reshape(B * H, S, D)
    em_raw = np.concatenate(
        [res.results[i]["emn"] for i in range(NCORES)], axis=0
    )
    band = _extract_band(em_raw).astype(np.float32)
    rn = 1.0 / band.sum(axis=2, keepdims=True)
    probs = band * rn
    ctx = (ctx_raw * rn).reshape(B, H, S, D)
    return ctx, probs
